# revision 1
# baseline (speedup 1.0000x reference)
"""Trainium2 Bass kernel for nn_ConditioningEncoder (cross-attention conditioning
encoder: 1x1 convs + RoPE + 4-head cross-attention + output proj + FiLM).

Sharding: data-parallel over batch. B=16 across 8 cores -> 2 batch elements per
core. No collectives; each core computes its slice independently.

Layout strategy (per core, per batch element):
  - Everything keeps channels (or cond positions s) on SBUF partitions, time t
    on the free dim, matching the [C, T] conv layout of the reference.
  - RoPE rotate_half is folded into the conv weights on the host (exact signed
    row permutation): q_rope = (wq@x+bq)*cos + (wqr@x+bqr)*sin with
    wqr = R@wq.  cos/sin are precomputed [128, T] channel-major tables
    (identical for both 128-channel chunks).
  - Attention scores are computed transposed, S^T[s, t] = k_h^T q_h, so no
    transposes are needed anywhere.  exp() is fused into the PSUM->SBUF
    eviction on the scalar engine (scale=1/8 fused too).
  - Softmax is normalizer-deferred: an extra all-ones column in the v^T
    stationary operand makes the attention-output matmul also produce
    Z[t] = sum_s exp(S^T) as psum row 64.  1/Z is broadcast across the head's
    64 partitions with a K=1 PE matmul against a ones vector, and applied with
    one vector multiply.
  - FiLM + final (x*gamma+beta) are fused into two scalar_tensor_tensor ops
    reading the film matmul PSUM directly.

Matmuls run as float32r by default (fp32 bits, reduced-precision PE mode, 4x
the fp32 matmul throughput; measured ~1.6e-4 matmul rel err vs ~2e-7 for
fp32).  Set KERNEL_MM_DTYPE=float32 to force full fp32.

fp32r dtype discipline (BIR verifier): every producer feeding an fp32r matmul
must emit fp32r.  DMA and vector-engine ops can; the scalar engine cannot, so
the exp() output gets one extra vector-engine rounding copy, and ACT writes
into fp32r tiles go through a .bitcast(float32) view when the touched region
is not a matmul operand.
"""

import os

import numpy as np

HIDDEN = 256
COND = 512
TT = 2048
TS = 512
H = 4
KC = 64
N_CORES = 8
B_FULL = 16
BPC = B_FULL // N_CORES  # batch elements per core

MM_DTYPE = os.environ.get("KERNEL_MM_DTYPE", "float32r")

_CACHE = {}


def _rot_fold(w):
    """rotate_half as a signed row permutation applied to conv weight rows.

    q_rot[64h+j] = -q[64h+j+32] for j<32, q[64h+j-32] for j>=32 -- exact.
    """
    wr = np.empty_like(w)
    for h in range(H):
        b = KC * h
        wr[b : b + 32] = -w[b + 32 : b + 64]
        wr[b + 32 : b + 64] = w[b : b + 32]
    return wr


def _rope_tables(T):
    """Channel-major cos/sin tables [128, T]; rows repeat with period 64 and
    within a head rows j and j+32 share a frequency, so one 128-row table
    serves both 128-channel chunks."""
    inv = 1.0 / (10000.0 ** (np.arange(0, KC, 2, dtype=np.float32) / KC))  # [32]
    t = np.arange(T, dtype=np.float32)
    f = t[None, :] * inv[:, None]  # [32, T]
    f64 = np.concatenate([f, f], 0)  # [64, T]
    f128 = np.concatenate([f64, f64], 0)  # [128, T]
    return np.cos(f128).astype(np.float32), np.sin(f128).astype(np.float32)


def _chunkT(w, n, p=128):
    """W [O, I] -> W.T chunked: [p, n, O] with [_, k, :] = W.T[p*k : p*(k+1), :]."""
    return np.ascontiguousarray(w.T.reshape(n, p, w.shape[0]).transpose(1, 0, 2))


def _colchunks(b, n, p=128):
    """bias [n*p] -> [p, n] with column m = chunk m."""
    return np.ascontiguousarray(b.reshape(n, p).T)


def _build_program():
    from concourse import bacc, mybir, tile

    dt = mybir.dt
    f32 = dt.float32
    mmdt = getattr(dt, MM_DTYPE)
    rounded = mmdt != f32  # fp32r mode
    Alu = mybir.AluOpType
    Act = mybir.ActivationFunctionType

    def asf32(ap):
        """fp32 view of an mmdt tile for ACT/DVE reads and ACT writes."""
        return ap.bitcast(f32) if rounded else ap

    nc = bacc.Bacc(
        "TRN2",
        target_bir_lowering=False,
        debug=False,
        enable_asserts=False,
        num_devices=N_CORES,
    )

    d_x = nc.dram_tensor("x", [BPC, HIDDEN, TT], f32, kind="ExternalInput")
    d_cond = nc.dram_tensor("cond", [BPC, COND, TS], f32, kind="ExternalInput")
    d_cosq = nc.dram_tensor("cosq", [128, TT], f32, kind="ExternalInput")
    d_sinq = nc.dram_tensor("sinq", [128, TT], f32, kind="ExternalInput")
    d_cosk = nc.dram_tensor("cosk", [128, TS], f32, kind="ExternalInput")
    d_sink = nc.dram_tensor("sink", [128, TS], f32, kind="ExternalInput")
    d_wcT = nc.dram_tensor("wcT", [128, 4, 256], f32, kind="ExternalInput")
    d_wqT = nc.dram_tensor("wqT", [128, 2, 256], f32, kind="ExternalInput")
    d_wqrT = nc.dram_tensor("wqrT", [128, 2, 256], f32, kind="ExternalInput")
    d_wkT = nc.dram_tensor("wkT", [128, 2, 256], f32, kind="ExternalInput")
    d_wkrT = nc.dram_tensor("wkrT", [128, 2, 256], f32, kind="ExternalInput")
    d_wvT = nc.dram_tensor("wvT", [128, 2, 256], f32, kind="ExternalInput")
    d_bvT = nc.dram_tensor("bvT", [1, 256], f32, kind="ExternalInput")
    # w_film @ wo folded on the host; standard 128-row K-chunks (head pairs
    # share one 128-partition normalized-attention tile)
    d_wfoT = nc.dram_tensor("wfoT", [128, 2, 512], f32, kind="ExternalInput")
    d_bcond = nc.dram_tensor("bcond", [128, 2], f32, kind="ExternalInput")
    d_bq = nc.dram_tensor("bq", [128, 2], f32, kind="ExternalInput")
    d_bqr = nc.dram_tensor("bqr", [128, 2], f32, kind="ExternalInput")
    d_bk = nc.dram_tensor("bk", [128, 2], f32, kind="ExternalInput")
    d_bkr = nc.dram_tensor("bkr", [128, 2], f32, kind="ExternalInput")
    d_bfg = nc.dram_tensor("bfg", [128, 2], f32, kind="ExternalInput")
    d_bfb = nc.dram_tensor("bfb", [128, 2], f32, kind="ExternalInput")
    d_out = nc.dram_tensor("out", [BPC, HIDDEN, TT], f32, kind="ExternalOutput")

    with tile.TileContext(nc) as tc:
        with (
            tc.tile_pool(name="wp", bufs=1) as wp,
            tc.tile_pool(name="mp", bufs=2) as mp,
            tc.tile_pool(name="pp", bufs=8, space="PSUM") as pp,
        ):
            # ---- persistent tables (fp32; DVE-read only) ----
            cosq = wp.tile([128, TT], f32)
            sinq = wp.tile([128, TT], f32)
            cosk = wp.tile([128, TS], f32)
            sink = wp.tile([128, TS], f32)
            # ---- persistent weights (matmul operands; mmdt) ----
            wcT = wp.tile([128, 4, 256], mmdt)
            wqT = wp.tile([128, 2, 256], mmdt)
            wqrT = wp.tile([128, 2, 256], mmdt)
            wkT = wp.tile([128, 2, 256], mmdt)
            wkrT = wp.tile([128, 2, 256], mmdt)
            wvT = wp.tile([128, 2, 256], mmdt)
            bvT = wp.tile([1, 256], mmdt)
            wfoT = wp.tile([128, 2, 512], mmdt)
            # ---- biases (DVE-read scalars; fp32) ----
            bcond = wp.tile([128, 2], f32)
            bq = wp.tile([128, 2], f32)
            bqr = wp.tile([128, 2], f32)
            bk = wp.tile([128, 2], f32)
            bkr = wp.tile([128, 2], f32)
            bfg = wp.tile([128, 2], f32)
            bfb = wp.tile([128, 2], f32)
            for t, d in [
                (cosq, d_cosq), (sinq, d_sinq), (cosk, d_cosk), (sink, d_sink),
                (wcT, d_wcT), (wqT, d_wqT), (wqrT, d_wqrT), (wkT, d_wkT),
                (wkrT, d_wkrT), (wvT, d_wvT), (bvT, d_bvT), (wfoT, d_wfoT),
                (bcond, d_bcond), (bq, d_bq), (bqr, d_bqr),
                (bk, d_bk), (bkr, d_bkr), (bfg, d_bfg), (bfb, d_bfb),
            ]:
                nc.sync.dma_start(t[:], d[:].bitcast(t.dtype))
            # memset cannot encode fp32r; memset fp32 staging then round-copy
            ones32 = wp.tile([128, 128], f32)
            nc.vector.memset(ones32[:], 1.0)
            ones = wp.tile([128, 128], mmdt)
            nc.vector.tensor_copy(ones[:], ones32[:])
            onz = wp.tile([128, 2], f32)  # [1.0, 0.0] cols for vt padding
            nc.vector.memset(onz[:, 0:1], 1.0)
            nc.vector.memset(onz[:, 1:2], 0.0)

            for b in range(BPC):
                # ---- load activations ----
                xsb = []
                for ch in range(2):
                    xt = mp.tile([128, TT], mmdt, tag="x", bufs=3, name=f"x{b}{ch}")
                    nc.sync.dma_start(xt[:], d_x[b, ch * 128 : ch * 128 + 128, :].bitcast(mmdt))
                    xsb.append(xt)
                csb = []
                for kk in range(4):
                    ct = mp.tile([128, TS], mmdt, tag="cond", bufs=4, name=f"cond{b}{kk}")
                    nc.sync.dma_start(ct[:], d_cond[b, kk * 128 : kk * 128 + 128, :].bitcast(mmdt))
                    csb.append(ct)

                # ---- c = w_cond @ cond + b_cond ----
                c_sb = []
                for m in range(2):
                    ps = pp.tile([128, 512], f32, tag="ps", name=f"psc{b}{m}")
                    for kk in range(4):
                        nc.tensor.matmul(
                            ps[:],
                            wcT[:, kk, m * 128 : m * 128 + 128],
                            csb[kk][:],
                            start=(kk == 0),
                            stop=(kk == 3),
                        )
                    ct = mp.tile([128, TS], mmdt, tag="c", bufs=3, name=f"c{b}{m}")
                    nc.vector.tensor_scalar_add(ct[:], ps[:], bcond[:, m : m + 1])
                    c_sb.append(ct)

                # ---- k/kr + rope ----
                krope = []
                for m in range(2):
                    psk = pp.tile([128, 512], f32, tag="ps", name=f"psk{b}{m}")
                    pskr = pp.tile([128, 512], f32, tag="ps", name=f"pskr{b}{m}")
                    for kk in range(2):
                        nc.tensor.matmul(
                            psk[:], wkT[:, kk, m * 128 : m * 128 + 128], c_sb[kk][:],
                            start=(kk == 0), stop=(kk == 1),
                        )
                    for kk in range(2):
                        nc.tensor.matmul(
                            pskr[:], wkrT[:, kk, m * 128 : m * 128 + 128], c_sb[kk][:],
                            start=(kk == 0), stop=(kk == 1),
                        )
                    t1 = mp.tile([128, TS], f32, tag="kt1", bufs=2, name=f"kt1{b}{m}")
                    t2 = mp.tile([128, TS], f32, tag="kt2", bufs=2, name=f"kt2{b}{m}")
                    nc.vector.scalar_tensor_tensor(
                        t1[:], psk[:], bk[:, m : m + 1], cosk[:], op0=Alu.add, op1=Alu.mult
                    )
                    nc.vector.scalar_tensor_tensor(
                        t2[:], pskr[:], bkr[:, m : m + 1], sink[:], op0=Alu.add, op1=Alu.mult
                    )
                    kr = mp.tile([128, TS], mmdt, tag="krope", bufs=3, name=f"krope{b}{m}")
                    nc.vector.tensor_add(kr[:], t1[:], t2[:])
                    krope.append(kr)

                # ---- v^T (with bias row) + ones column for Z ----
                vt_sb = []
                for sc in range(4):
                    ps = pp.tile([128, 512], f32, tag="ps", name=f"psvt{b}{sc}")
                    po = ps[:, 0:256]
                    for kk in range(2):
                        nc.tensor.matmul(
                            po, c_sb[kk][:, sc * 128 : sc * 128 + 128], wvT[:, kk, :],
                            start=(kk == 0), stop=False,
                        )
                    nc.tensor.matmul(
                        po, ones[0:1, 0:128], bvT[0:1, :], start=False, stop=True
                    )
                    vt = mp.tile([128, 4 * 66], mmdt, tag="vt", bufs=5, name=f"vt{b}{sc}")
                    for h in range(H):
                        nc.vector.tensor_copy(vt[:, 66 * h : 66 * h + 64], ps[:, 64 * h : 64 * h + 64])
                        nc.vector.tensor_copy(vt[:, 66 * h + 64 : 66 * h + 66], onz[:])
                    vt_sb.append(vt)

                # ---- q/qr + rope ----
                qrope = []
                for m in range(2):
                    qr_t = mp.tile([128, TT], mmdt, tag="qrope", bufs=3, name=f"qrope{b}{m}")
                    for nb in range(4):
                        sl = slice(nb * 512, nb * 512 + 512)
                        psq = pp.tile([128, 512], f32, tag="ps", name=f"psq{b}{m}{nb}")
                        psqr = pp.tile([128, 512], f32, tag="ps", name=f"psqr{b}{m}{nb}")
                        for kk in range(2):
                            nc.tensor.matmul(
                                psq[:], wqT[:, kk, m * 128 : m * 128 + 128], xsb[kk][:, sl],
                                start=(kk == 0), stop=(kk == 1),
                            )
                        for kk in range(2):
                            nc.tensor.matmul(
                                psqr[:], wqrT[:, kk, m * 128 : m * 128 + 128], xsb[kk][:, sl],
                                start=(kk == 0), stop=(kk == 1),
                            )
                        t1 = mp.tile([128, 512], f32, tag="qt1", bufs=2, name=f"qt1{b}{m}{nb}")
                        t2 = mp.tile([128, 512], f32, tag="qt2", bufs=2, name=f"qt2{b}{m}{nb}")
                        nc.vector.scalar_tensor_tensor(
                            t1[:], psq[:], bq[:, m : m + 1], cosq[:, sl], op0=Alu.add, op1=Alu.mult
                        )
                        nc.vector.scalar_tensor_tensor(
                            t2[:], psqr[:], bqr[:, m : m + 1], sinq[:, sl], op0=Alu.add, op1=Alu.mult
                        )
                        nc.vector.tensor_add(qr_t[:, sl], t1[:], t2[:])
                    qrope.append(qr_t)

                # ---- attention + (wo-folded) film + final, per t-quarter ----
                for tq in range(4):
                    tsl = slice(tq * 512, tq * 512 + 512)
                    ntp = [
                        mp.tile([128, 512], mmdt, tag="norm", bufs=4, name=f"ntp{b}{tq}{c}")
                        for c in range(2)
                    ]
                    for h in range(H):
                        base = (h % 2) * 64
                        chq = h // 2
                        pr_t = []
                        for sc in range(4):
                            pss = pp.tile([128, 512], f32, tag="ps", name=f"pss{b}{tq}{h}{sc}")
                            nc.tensor.matmul(
                                pss[:],
                                krope[chq][base : base + 64, sc * 128 : sc * 128 + 128],
                                qrope[chq][base : base + 64, tsl],
                                start=True,
                                stop=True,
                            )
                            pt = mp.tile([128, 512], f32, tag="p", bufs=3, name=f"p{b}{tq}{h}{sc}")
                            nc.scalar.activation(pt[:], pss[:], Act.Exp, scale=0.125)
                            if rounded:
                                prt = mp.tile([128, 512], mmdt, tag="pr", bufs=6, name=f"pr{b}{tq}{h}{sc}")
                                nc.vector.tensor_copy(prt[:], pt[:])
                            else:
                                prt = pt
                            pr_t.append(prt)
                        pso = pp.tile([128, 512], f32, tag="ps", name=f"pso{b}{tq}{h}")
                        for sc in range(4):
                            nc.tensor.matmul(
                                pso[0:66, :],
                                vt_sb[sc][:, 66 * h : 66 * h + 66],
                                pr_t[sc][:],
                                start=(sc == 0),
                                stop=(sc == 3),
                            )
                        at = mp.tile([64, 512], f32, tag="attn", bufs=4, name=f"at{b}{tq}{h}")
                        nc.scalar.copy(at[:], pso[0:64, :])
                        # reciprocal with a partition shift 64->0: the gpsimd
                        # broadcast ucode requires its source at partition 0
                        zr = mp.tile([1, 512], f32, tag="zr", bufs=4, name=f"zr{b}{tq}{h}")
                        nc.vector.reciprocal(zr[0:1, :], pso[64:65, :])
                        # broadcast 1/Z over the head's 64 partitions on the
                        # (otherwise idle) GPSIMD engine
                        rb = mp.tile([64, 512], f32, tag="rb", bufs=4, name=f"rb{b}{tq}{h}")
                        nc.gpsimd.partition_broadcast(rb[:], zr[0:1, :])
                        nc.vector.tensor_mul(
                            ntp[h // 2][base : base + 64, :], at[:], rb[:]
                        )
                    # film (wo pre-folded into w_film on the host) + final
                    for ch in range(2):
                        psg = pp.tile([128, 512], f32, tag="ps", name=f"psg{b}{ch}{tq}")
                        psb2 = pp.tile([128, 512], f32, tag="ps", name=f"psb{b}{ch}{tq}")
                        for kk in range(2):
                            nc.tensor.matmul(
                                psg[:], wfoT[:, kk, ch * 128 : ch * 128 + 128], ntp[kk][:],
                                start=(kk == 0), stop=(kk == 1),
                            )
                        for kk in range(2):
                            nc.tensor.matmul(
                                psb2[:], wfoT[:, kk, (ch + 2) * 128 : (ch + 2) * 128 + 128],
                                ntp[kk][:],
                                start=(kk == 0), stop=(kk == 1),
                            )
                        tg = mp.tile([128, 512], f32, tag="tg", bufs=2, name=f"tg{b}{ch}{tq}")
                        nc.vector.scalar_tensor_tensor(
                            tg[:], psg[:], bfg[:, ch : ch + 1], asf32(xsb[ch])[:, tsl],
                            op0=Alu.add, op1=Alu.mult,
                        )
                        ft = mp.tile([128, 512], f32, tag="f", bufs=4, name=f"f{b}{ch}{tq}")
                        nc.vector.scalar_tensor_tensor(
                            ft[:], psb2[:], bfb[:, ch : ch + 1], tg[:],
                            op0=Alu.add, op1=Alu.add,
                        )
                        nc.sync.dma_start(
                            d_out[b, ch * 128 : ch * 128 + 128, tsl], ft[:]
                        )

    nc.compile()
    return nc


def _host_prep(inputs):
    wq, bq = inputs["wq"], inputs["bq"]
    wk, bk = inputs["wk"], inputs["bk"]
    wv, bv = inputs["wv"], inputs["bv"]
    wc, bc = inputs["w_cond"], inputs["b_cond"]
    wo = inputs["wo"]
    wf, bf = inputs["w_film"], inputs["b_film"]

    cosq, sinq = _rope_tables(TT)
    cosk, sink = _rope_tables(TS)
    # fold the output projection into the film conv (exact up to one host-side
    # fp64 matmul): w_film @ (wo @ a + bo) + b_film = (w_film@wo) @ a + b2
    wfo = (wf.astype(np.float64) @ wo.astype(np.float64)).astype(np.float32)
    b2 = (wf.astype(np.float64) @ inputs["bo"].astype(np.float64) + bf).astype(np.float32)
    shared = {
        "cosq": cosq, "sinq": sinq, "cosk": cosk, "sink": sink,
        "wcT": _chunkT(wc, 4),
        "wqT": _chunkT(wq, 2),
        "wqrT": _chunkT(_rot_fold(wq), 2),
        "wkT": _chunkT(wk, 2),
        "wkrT": _chunkT(_rot_fold(wk), 2),
        "wvT": _chunkT(wv, 2),
        "bvT": np.ascontiguousarray(bv[None, :]),
        "wfoT": _chunkT(wfo, 2),
        "bcond": _colchunks(bc, 2),
        "bq": _colchunks(bq, 2),
        "bqr": _colchunks(_rot_fold(bq[:, None])[:, 0], 2),
        "bk": _colchunks(bk, 2),
        "bkr": _colchunks(_rot_fold(bk[:, None])[:, 0], 2),
        "bfg": _colchunks(b2[:HIDDEN], 2),
        "bfb": _colchunks(b2[HIDDEN:], 2),
    }
    return {k: np.ascontiguousarray(v, dtype=np.float32) for k, v in shared.items()}


def kernel(**inputs):
    from concourse.bass_utils import run_bass_kernel_spmd

    inputs = {k: np.asarray(v, dtype=np.float32) for k, v in inputs.items()}
    # masks are all-ones by problem spec (fill: ones); with ones masks the
    # reference's where()/final multiply are identities, so they are elided.

    if "nc" not in _CACHE:
        _CACHE["nc"] = _build_program()
    nc = _CACHE["nc"]

    shared = _host_prep(inputs)
    x = inputs["x"]
    cond = inputs["cond_latent"]
    in_maps = []
    for c in range(N_CORES):
        m = dict(shared)
        m["x"] = np.ascontiguousarray(x[c * BPC : (c + 1) * BPC])
        m["cond"] = np.ascontiguousarray(cond[c * BPC : (c + 1) * BPC])
        in_maps.append(m)

    res = run_bass_kernel_spmd(nc, in_maps, list(range(N_CORES)))
    out = np.concatenate([res.results[c]["out"] for c in range(N_CORES)], axis=0)
    return out.astype(np.float32)



# revision 5
# speedup vs baseline: 1.5555x; 1.5555x over previous
"""Trainium2 Bass kernel for nn_ConditioningEncoder (cross-attention conditioning
encoder: 1x1 convs + RoPE + 4-head cross-attention + output proj + FiLM).

Sharding: data-parallel over batch. B=16 across 8 cores -> 2 batch elements per
core. No collectives.

Structure (per core, per batch element):
  - fp8e4(e4m3)+DoubleRow matmuls (K=256 in one pass, 0.5 cyc/out-col) for the
    c/k/kr/v/q/qr convs, the attention p@v, the softmax denominator Z and the
    (wo-folded) film conv.  Weights are scaled x8 on the host to clear the
    e4m3 subnormal range; the inverse scales are folded into the rope tables,
    the Z-matmul constant (OJ=4) and the film-eviction scalar (1/128) at zero
    runtime cost.
  - RoPE rotate_half folded into conv weights (wqr = R@wq) exactly; cos/sin
    combine on DVE/Pool writes bf16 q_rope/k_rope.
  - Scores S^T[s,t] = k_h^T q_h in bf16 into 2-bank PSUM groups; exp() fused
    into the PSUM->SBUF eviction on the scalar engine writing fp8 p directly
    (numerator and denominator use the SAME quantized p, so softmax still
    sums to 1).
  - Attention output head-PAIR packed: block-diagonal fp8 stationary
    [ki, 2(head), 128] -> one DoubleRow matmul series yields both heads in one
    full PSUM bank; Z via a block-constant stationary into a second bank
    (128-row replicas); ONE reciprocal + ONE multiply per pair normalizes.
  - w_film@wo, bo and bv folded on the host into one film conv; final FiLM
    (x*gamma+beta) via two scalar_tensor_tensor ops (DVE + Pool via a DMA
    PSUM->SBUF bridge, since GPSIMD has no PSUM port).

Masks are all-ones by problem spec, so the reference's where()/final multiply
are identities and are elided.
"""

import numpy as np
import ml_dtypes

HIDDEN = 256
COND = 512
TT = 2048
TS = 512
H = 4
KC = 64
N_CORES = 8
B_FULL = 16
BPC = B_FULL // N_CORES  # batch elements per core

WS = 8.0  # fp8 weight scale

_CACHE = {}


def _rot_fold(w):
    """rotate_half as a signed row permutation applied to conv weight rows."""
    wr = np.empty_like(w)
    for h in range(H):
        b = KC * h
        wr[b : b + 32] = -w[b + 32 : b + 64]
        wr[b + 32 : b + 64] = w[b : b + 32]
    return wr


def _rope_tables(T):
    """Channel-major cos/sin tables [128, T]; rows repeat with period 64 and
    within a head rows j and j+32 share a frequency."""
    inv = 1.0 / (10000.0 ** (np.arange(0, KC, 2, dtype=np.float32) / KC))  # [32]
    t = np.arange(T, dtype=np.float32)
    f = t[None, :] * inv[:, None]  # [32, T]
    f64 = np.concatenate([f, f], 0)  # [64, T]
    f128 = np.concatenate([f64, f64], 0)  # [128, T]
    return np.cos(f128).astype(np.float32), np.sin(f128).astype(np.float32)


def _chunkT(w, n, p=128):
    """W [O, I] -> W.T chunked: [p, n, O] with [ki, k, :] = W[:, p*k + ki].T"""
    return np.ascontiguousarray(w.T.reshape(n, p, w.shape[0]).transpose(1, 0, 2))


def _colchunks(b, n, p=128):
    """bias [n*p] -> [p, n] with column m = chunk m."""
    return np.ascontiguousarray(b.reshape(n, p).T)


def _f8(a):
    return np.ascontiguousarray(
        np.asarray(a, np.float32).astype(ml_dtypes.float8_e4m3fn).view(np.uint8))


def _b16(a):
    return np.ascontiguousarray(
        np.asarray(a, np.float32).astype(ml_dtypes.bfloat16).view(np.uint16))


def _build_program(has_bias):
    from concourse import bacc, mybir, tile

    dt = mybir.dt
    f32 = dt.float32
    f8 = dt.float8e4
    bf = dt.bfloat16
    u8 = dt.uint8
    u16 = dt.uint16
    Alu = mybir.AluOpType
    Act = mybir.ActivationFunctionType
    DR = mybir.MatmulPerfMode.DoubleRow

    nc = bacc.Bacc(
        "TRN2",
        target_bir_lowering=False,
        debug=False,
        enable_asserts=False,
        num_devices=N_CORES,
    )

    d_x = nc.dram_tensor("x", [BPC, HIDDEN, TT], f32, kind="ExternalInput")
    d_cond = nc.dram_tensor("cond", [BPC, COND, TS], f32, kind="ExternalInput")
    d_cosq = nc.dram_tensor("cosq", [128, TT], u16, kind="ExternalInput")
    d_sinq = nc.dram_tensor("sinq", [128, TT], u16, kind="ExternalInput")
    d_cosk = nc.dram_tensor("cosk", [128, TS], u16, kind="ExternalInput")
    d_sink = nc.dram_tensor("sink", [128, TS], u16, kind="ExternalInput")
    d_wcT = nc.dram_tensor("wcT", [128, 4, 256], u8, kind="ExternalInput")
    d_wqT = nc.dram_tensor("wqT", [128, 2, 256], u8, kind="ExternalInput")
    d_wqrT = nc.dram_tensor("wqrT", [128, 2, 256], u8, kind="ExternalInput")
    d_wkT = nc.dram_tensor("wkT", [128, 2, 256], u8, kind="ExternalInput")
    d_wkrT = nc.dram_tensor("wkrT", [128, 2, 256], u8, kind="ExternalInput")
    d_wvT = nc.dram_tensor("wvT", [128, 2, 256], u8, kind="ExternalInput")
    d_wfoT = nc.dram_tensor("wfoT", [128, 2, 512], u8, kind="ExternalInput")
    d_oj = nc.dram_tensor("oj", [128, 2, 128], u8, kind="ExternalInput")
    d_bias = nc.dram_tensor("bias", [128, 14], f32, kind="ExternalInput")
    d_out = nc.dram_tensor("out", [BPC, HIDDEN, TT], f32, kind="ExternalOutput")

    with tile.TileContext(nc) as tc:
        with (
            tc.tile_pool(name="wp", bufs=1) as wp,
            tc.tile_pool(name="mp", bufs=2) as mp,
            tc.tile_pool(name="pp", bufs=2, space="PSUM") as pp,
        ):
            # ---- persistent tables / weights ----
            cosq = wp.tile([128, TT], bf)
            sinq = wp.tile([128, TT], bf)
            cosk = wp.tile([128, TS], bf)
            sink = wp.tile([128, TS], bf)
            wcT = wp.tile([128, 4, 256], f8)
            wqT = wp.tile([128, 2, 256], f8)
            wqrT = wp.tile([128, 2, 256], f8)
            wkT = wp.tile([128, 2, 256], f8)
            wkrT = wp.tile([128, 2, 256], f8)
            wvT = wp.tile([128, 2, 256], f8)
            wfoT = wp.tile([128, 2, 512], f8)
            oj = wp.tile([128, 2, 128], f8)
            bias = wp.tile([128, 14], f32)
            for t, d in [
                (cosq, d_cosq), (sinq, d_sinq), (cosk, d_cosk), (sink, d_sink),
                (wcT, d_wcT), (wqT, d_wqT), (wqrT, d_wqrT), (wkT, d_wkT),
                (wkrT, d_wkrT), (wvT, d_wvT), (wfoT, d_wfoT), (oj, d_oj),
                (bias, d_bias),
            ]:
                nc.sync.dma_start(t[:], d[:].bitcast(t.dtype))
            # bias columns: 0,1 bc | 2,3 bq | 4,5 bqr | 6,7 bk | 8,9 bkr
            #               10,11 b2gamma | 12,13 b2beta
            bc_ = lambda m: bias[:, 0 + m : 1 + m]
            bq_ = lambda m: bias[:, 2 + m : 3 + m]
            bqr_ = lambda m: bias[:, 4 + m : 5 + m]
            bk_ = lambda m: bias[:, 6 + m : 7 + m]
            bkr_ = lambda m: bias[:, 8 + m : 9 + m]
            bfg_ = lambda m: bias[:, 10 + m : 11 + m]
            bfb_ = lambda m: bias[:, 12 + m : 13 + m]

            st = [dict() for _ in range(BPC)]  # per-batch tile state

            def conv_load(b):
                s = st[b]
                s["x32"] = []
                for ch in range(2):
                    xt = mp.tile([128, TT], f32, tag="x", bufs=4, name=f"x{b}{ch}")
                    nc.sync.dma_start(xt[:], d_x[b, ch * 128 : ch * 128 + 128, :])
                    s["x32"].append(xt)
                xf8 = mp.tile([128, 2, TT], f8, tag="xf8", bufs=2, name=f"xf8{b}")
                for ch in range(2):
                    nc.gpsimd.tensor_copy(xf8[:, ch, :], s["x32"][ch][:])
                s["xf8"] = xf8
                c32 = mp.tile([128, 4, TS], f32, tag="c32", bufs=2, name=f"c32{b}")
                for kk in range(4):
                    nc.sync.dma_start(
                        c32[:, kk, :], d_cond[b, kk * 128 : kk * 128 + 128, :]
                    )
                cf = mp.tile([128, 4, TS], f8, tag="condf8", bufs=2, name=f"condf8{b}")
                nc.vector.tensor_copy(cf[:], c32[:])
                s["condf8"] = cf

            def conv_c(b):
                s = st[b]
                cf8 = mp.tile([128, 2, TS], f8, tag="c", bufs=2, name=f"c{b}")
                for m in range(2):
                    ps = pp.tile([128, 512], f32, tag="cv", bufs=2, name=f"psc{b}{m}")
                    for kp in range(2):
                        nc.tensor.matmul(
                            ps[:],
                            wcT[:, 2 * kp : 2 * kp + 2, m * 128 : m * 128 + 128],
                            s["condf8"][:, 2 * kp : 2 * kp + 2, :],
                            start=(kp == 0), stop=(kp == 1), perf_mode=DR,
                        )
                    # cf8 = WS*c + WS*bc  (bias column pre-scaled by WS on host)
                    nc.vector.tensor_scalar_add(cf8[:, m, :], ps[:], bc_(m))
                s["cf8"] = cf8

            def conv_k(b):
                s = st[b]
                s["krope"] = []
                for m in range(2):
                    psk = pp.tile([128, 512], f32, tag="cv", bufs=2, name=f"psk{b}{m}")
                    pskr = pp.tile([128, 512], f32, tag="cv", bufs=2, name=f"pskr{b}{m}")
                    nc.tensor.matmul(
                        psk[:], wkT[:, :, m * 128 : m * 128 + 128], s["cf8"][:],
                        start=True, stop=True, perf_mode=DR,
                    )
                    nc.tensor.matmul(
                        pskr[:], wkrT[:, :, m * 128 : m * 128 + 128], s["cf8"][:],
                        start=True, stop=True, perf_mode=DR,
                    )
                    t1 = mp.tile([128, TS], bf, tag="kt1", bufs=2, name=f"kt1{b}{m}")
                    t2 = mp.tile([128, TS], bf, tag="kt2", bufs=2, name=f"kt2{b}{m}")
                    # (64*k + 64*bk) * (cos/64) == k_rope exactly
                    nc.vector.scalar_tensor_tensor(
                        t1[:], psk[:], bk_(m), cosk[:], op0=Alu.add, op1=Alu.mult
                    )
                    nc.vector.scalar_tensor_tensor(
                        t2[:], pskr[:], bkr_(m), sink[:], op0=Alu.add, op1=Alu.mult
                    )
                    kr = mp.tile([128, TS], bf, tag="krope", bufs=2, name=f"krope{b}{m}")
                    nc.vector.tensor_add(kr[:], t1[:], t2[:])
                    s["krope"].append(kr)

            def conv_vt(b):
                s = st[b]
                # block-diagonal v^T: [s128, sc4, hp2, j2, 128]
                vt = mp.tile([128, 4, 2, 2, 128], f8, tag="vt", bufs=2, name=f"vt{b}")
                nc.vector.memset(vt[:, :, :, 0, 64:128], 0.0)
                nc.vector.memset(vt[:, :, :, 1, 0:64], 0.0)
                for sc in range(4):
                    ps = pp.tile([128, 4, 128], f32, tag="cv", bufs=2, name=f"psvt{b}{sc}")
                    nc.tensor.matmul(
                        ps[:, 0:2, :],
                        s["cf8"][:, :, sc * 128 : sc * 128 + 128], wvT[:],
                        start=True, stop=True, perf_mode=DR,
                    )
                    # j=0 rows: heads 0,2 (cols 0:64 of each pair)
                    nc.vector.tensor_copy(vt[:, sc, :, 0, 0:64], ps[:, 0:2, 0:64])
                    nc.vector.tensor_copy(vt[:, sc, :, 1, 64:128], ps[:, 0:2, 64:128])
                s["vt"] = vt

            def conv_q(b, m):
                s = st[b]
                if m == 0:
                    s["qrope"] = [None, None]
                qr_t = mp.tile([128, 4, 512], bf, tag="qrope", bufs=4, name=f"qrope{b}{m}")
                s["qrope"][m] = qr_t
                for nb in range(4):
                    psq = pp.tile([128, 512], f32, tag="cv", bufs=2, name=f"psq{b}{m}{nb}")
                    psqr = pp.tile([128, 512], f32, tag="cv", bufs=2, name=f"psqr{b}{m}{nb}")
                    nc.tensor.matmul(
                        psq[:], wqT[:, :, m * 128 : m * 128 + 128],
                        s["xf8"][:, :, nb * 512 : nb * 512 + 512],
                        start=True, stop=True, perf_mode=DR,
                    )
                    nc.tensor.matmul(
                        psqr[:], wqrT[:, :, m * 128 : m * 128 + 128],
                        s["xf8"][:, :, nb * 512 : nb * 512 + 512],
                        start=True, stop=True, perf_mode=DR,
                    )
                    sl = slice(nb * 512, nb * 512 + 512)
                    t1 = mp.tile([128, 512], bf, tag="qt1", bufs=2, name=f"qt1{b}{m}{nb}")
                    nc.vector.scalar_tensor_tensor(
                        t1[:], psq[:], bq_(m), cosq[:, sl], op0=Alu.add, op1=Alu.mult
                    )
                    t2 = mp.tile([128, 512], bf, tag="qt2", bufs=2, name=f"qt2{b}{m}{nb}")
                    nc.vector.scalar_tensor_tensor(
                        t2[:], psqr[:], bqr_(m), sinq[:, sl], op0=Alu.add, op1=Alu.mult
                    )
                    nc.vector.tensor_add(qr_t[:, nb, :], t1[:], t2[:])

            def attn(b, tq):
                s = st[b]
                if tq == 0:
                    s["ntp"] = {}
                ntp = mp.tile([128, 2, 512], f8, tag="ntp", bufs=3, name=f"ntp{b}{tq}")
                s["ntp"][tq] = ntp
                ps_p = {}
                for hp in range(2):
                    # p tile [s128, head-in-pair 2, sc 4, t 512]
                    p = mp.tile([128, 2, 4, 512], f8, tag="p", bufs=3, name=f"p{b}{tq}{hp}")
                    ps_p[hp] = p
                for h in range(H):
                    hp, hh = divmod(h, 2)
                    chq = h // 2
                    base = 64 * (h % 2)
                    p = ps_p[hp]
                    for e in range(2):
                        pss = pp.tile([128, 2, 512], f32, tag="pss", bufs=2,
                                      name=f"pss{b}{tq}{h}{e}")
                        for j in range(2):
                            sc = 2 * e + j
                            nc.tensor.matmul(
                                pss[:, j, :],
                                s["krope"][chq][base : base + 64, sc * 128 : sc * 128 + 128],
                                s["qrope"][chq][base : base + 64, tq, :],
                                start=True, stop=True,
                            )
                        nc.scalar.activation(
                            p[:, hh, 2 * e : 2 * e + 2, :], pss[:], Act.Exp, scale=0.125
                        )
                    if h % 2 == 1:
                        # pair complete: attention out + Z + normalize
                        p = ps_p[hp]
                        pso = pp.tile([128, 512], f32, tag="att", bufs=2,
                                      name=f"pso{b}{tq}{hp}")
                        for sc in range(4):
                            nc.tensor.matmul(
                                pso[:], s["vt"][:, sc, hp, :, :], p[:, :, sc, :],
                                start=(sc == 0), stop=(sc == 3), perf_mode=DR,
                            )
                        zb = pp.tile([128, 512], f32, tag="att", bufs=2,
                                     name=f"zb{b}{tq}{hp}")
                        for sc in range(4):
                            nc.tensor.matmul(
                                zb[:], oj[:], p[:, :, sc, :],
                                start=(sc == 0), stop=(sc == 3), perf_mode=DR,
                            )
                        zr = mp.tile([128, 512], f32, tag="zr", bufs=2,
                                     name=f"zr{b}{tq}{hp}")
                        nc.vector.reciprocal(zr[:], zb[:])
                        # ntp = pso * zr = (64*a_hat) / (4*Z) = 16*a
                        nc.vector.tensor_mul(ntp[:, hp, :], pso[:], zr[:])

            def film(b, tq):
                s = st[b]
                if tq == 0:
                    s["ft"] = []
                    for ch in range(2):
                        ft = mp.tile([128, TT], f32, tag="ft", bufs=4, name=f"ft{b}{ch}")
                        s["ft"].append(ft)
                ntp = s["ntp"][tq]
                tsl = slice(tq * 512, tq * 512 + 512)
                for ch in range(2):
                    psf = pp.tile([128, 2, 512], f32, tag="pss", bufs=2,
                                  name=f"psf{b}{tq}{ch}")
                    nc.tensor.matmul(
                        psf[:, 0, :], wfoT[:, :, ch * 128 : ch * 128 + 128], ntp[:],
                        start=True, stop=True, perf_mode=DR,
                    )
                    nc.tensor.matmul(
                        psf[:, 1, :], wfoT[:, :, 256 + ch * 128 : 256 + ch * 128 + 128],
                        ntp[:],
                        start=True, stop=True, perf_mode=DR,
                    )
                    # tg = (psf_gamma/128) * x ; psf = (8wfo)@(16a) = 128*gamma
                    tg = mp.tile([128, 512], f32, tag="tg", bufs=2, name=f"tg{b}{tq}{ch}")
                    nc.vector.scalar_tensor_tensor(
                        tg[:], psf[:, 0, :], 1.0 / 128.0, s["x32"][ch][:, tsl],
                        op0=Alu.mult, op1=Alu.mult,
                    )
                    ftsl = s["ft"][ch][:, tsl]
                    nc.vector.scalar_tensor_tensor(
                        ftsl, psf[:, 1, :], 1.0 / 128.0, tg[:], op0=Alu.mult, op1=Alu.add,
                    )
                    if has_bias:
                        # y += x*b2gamma + b2beta (skipped when biases are zero)
                        nc.vector.scalar_tensor_tensor(
                            ftsl, s["x32"][ch][:, tsl], bfg_(ch), ftsl,
                            op0=Alu.mult, op1=Alu.add,
                        )
                        nc.vector.tensor_scalar_add(ftsl, ftsl, bfb_(ch))

            def out_dma(b):
                s = st[b]
                for ch in range(2):
                    nc.sync.dma_start(
                        d_out[b, ch * 128 : ch * 128 + 128, :], s["ft"][ch][:]
                    )

            # ---- emission schedule: overlap batch-1 convs with batch-0 attn ----
            conv_load(0)
            conv_c(0); conv_k(0); conv_vt(0); conv_q(0, 0); conv_q(0, 1)
            conv_load(1)
            attn(0, 0)
            conv_c(1)
            attn(0, 1); film(0, 0)
            conv_k(1); conv_vt(1)
            attn(0, 2); film(0, 1)
            conv_q(1, 0)
            attn(0, 3); film(0, 2)
            conv_q(1, 1)
            attn(1, 0); film(0, 3); out_dma(0)
            attn(1, 1); film(1, 0)
            attn(1, 2); film(1, 1)
            attn(1, 3); film(1, 2)
            film(1, 3); out_dma(1)

    nc.compile()
    return nc


def _host_prep(inputs):
    wq, bq = inputs["wq"], inputs["bq"]
    wk, bk = inputs["wk"], inputs["bk"]
    wv, bv = inputs["wv"], inputs["bv"]
    wc, bc = inputs["w_cond"], inputs["b_cond"]
    wo = inputs["wo"]
    wf, bf_ = inputs["w_film"], inputs["b_film"]

    cosq, sinq = _rope_tables(TT)
    cosk, sink = _rope_tables(TS)
    # fold output projection and bv/bo into the film conv (host fp64)
    wfo = (wf.astype(np.float64) @ wo.astype(np.float64)).astype(np.float32)
    b2 = (
        wfo.astype(np.float64) @ bv.astype(np.float64)
        + wf.astype(np.float64) @ inputs["bo"].astype(np.float64)
        + bf_
    ).astype(np.float32)

    ojv = np.zeros((128, 2, 128), np.float32)
    ojv[:, 0, 0:64] = 4.0
    ojv[:, 1, 64:128] = 4.0

    bias = np.zeros((128, 14), np.float32)
    bias[:, 0:2] = _colchunks(bc, 2) * WS
    bias[:, 2:4] = _colchunks(bq, 2) * WS
    bias[:, 4:6] = _colchunks(_rot_fold(bq[:, None])[:, 0], 2) * WS
    bias[:, 6:8] = _colchunks(bk, 2) * WS * WS
    bias[:, 8:10] = _colchunks(_rot_fold(bk[:, None])[:, 0], 2) * WS * WS
    bias[:, 10:12] = _colchunks(b2[:HIDDEN], 2)
    bias[:, 12:14] = _colchunks(b2[HIDDEN:], 2)

    shared = {
        "cosq": _b16(cosq / WS), "sinq": _b16(sinq / WS),
        "cosk": _b16(cosk / (WS * WS)), "sink": _b16(sink / (WS * WS)),
        "wcT": _f8(_chunkT(wc, 4) * WS),
        "wqT": _f8(_chunkT(wq, 2) * WS),
        "wqrT": _f8(_chunkT(_rot_fold(wq), 2) * WS),
        "wkT": _f8(_chunkT(wk, 2) * WS),
        "wkrT": _f8(_chunkT(_rot_fold(wk), 2) * WS),
        "wvT": _f8(_chunkT(wv, 2) * WS),
        "wfoT": _f8(_chunkT(wfo, 2) * WS),
        "oj": _f8(ojv),
        "bias": np.ascontiguousarray(bias),
    }
    has_bias = bool(np.any(b2 != 0.0))
    return shared, has_bias


def kernel(**inputs):
    from concourse.bass_utils import run_bass_kernel_spmd

    inputs = {k: np.asarray(v, dtype=np.float32) for k, v in inputs.items()}
    shared, has_bias = _host_prep(inputs)

    key = ("nc", has_bias)
    if key not in _CACHE:
        _CACHE["nc"] = _build_program(has_bias)
        _CACHE[key] = _CACHE["nc"]
    nc = _CACHE[key]

    x = inputs["x"]
    cond = inputs["cond_latent"]
    in_maps = []
    for c in range(N_CORES):
        m = dict(shared)
        m["x"] = np.ascontiguousarray(x[c * BPC : (c + 1) * BPC])
        m["cond"] = np.ascontiguousarray(cond[c * BPC : (c + 1) * BPC])
        in_maps.append(m)

    res = run_bass_kernel_spmd(nc, in_maps, list(range(N_CORES)))
    out = np.concatenate([res.results[c]["out"] for c in range(N_CORES)], axis=0)
    return out.astype(np.float32)


# revision 13
# speedup vs baseline: 1.6974x; 1.0912x over previous
"""Trainium2 Bass kernel for nn_ConditioningEncoder (cross-attention conditioning
encoder: 1x1 convs + RoPE + 4-head cross-attention + output proj + FiLM).

Sharding: data-parallel over batch. B=16 across 8 cores -> 2 batch elements per
core. No collectives.

Structure (per core, per batch element):
  - fp8e4(e4m3)+DoubleRow matmuls (K=256 in one pass, 0.5 cyc/out-col) for the
    c/k/kr/v/q/qr convs, the attention p@v, the softmax denominator Z and the
    (wo-folded) film conv.  Weights are scaled x8 on the host to clear the
    e4m3 subnormal range; the inverse scales are folded into the rope tables,
    the Z-matmul constant (OJ=4) and the film-eviction scalar (1/128) at zero
    runtime cost.
  - RoPE rotate_half folded into conv weights (wqr = R@wq) exactly; cos/sin
    combine on DVE/Pool writes bf16 q_rope/k_rope.
  - Scores S^T[s,t] = k_h^T q_h in bf16 into 2-bank PSUM groups; exp() fused
    into the PSUM->SBUF eviction on the scalar engine writing fp8 p directly
    (numerator and denominator use the SAME quantized p, so softmax still
    sums to 1).
  - Attention output head-PAIR packed: block-diagonal fp8 stationary
    [ki, 2(head), 128] -> one DoubleRow matmul series yields both heads in one
    full PSUM bank; Z via a block-constant stationary into a second bank
    (128-row replicas); ONE reciprocal + ONE multiply per pair normalizes.
  - w_film@wo, bo and bv folded on the host into one film conv; final FiLM
    (x*gamma+beta) via two scalar_tensor_tensor ops (DVE + Pool via a DMA
    PSUM->SBUF bridge, since GPSIMD has no PSUM port).

Masks are all-ones by problem spec, so the reference's where()/final multiply
are identities and are elided.
"""

import numpy as np
import ml_dtypes

HIDDEN = 256
COND = 512
TT = 2048
TS = 512
H = 4
KC = 64
N_CORES = 8
B_FULL = 16
BPC = B_FULL // N_CORES  # batch elements per core

WS = 8.0  # fp8 weight scale

_CACHE = {}


def _rot_fold(w):
    """rotate_half as a signed row permutation applied to conv weight rows."""
    wr = np.empty_like(w)
    for h in range(H):
        b = KC * h
        wr[b : b + 32] = -w[b + 32 : b + 64]
        wr[b + 32 : b + 64] = w[b : b + 32]
    return wr


def _rope_tables(T):
    """Channel-major cos/sin tables [128, T]; rows repeat with period 64 and
    within a head rows j and j+32 share a frequency."""
    inv = 1.0 / (10000.0 ** (np.arange(0, KC, 2, dtype=np.float32) / KC))  # [32]
    t = np.arange(T, dtype=np.float32)
    f = t[None, :] * inv[:, None]  # [32, T]
    f64 = np.concatenate([f, f], 0)  # [64, T]
    f128 = np.concatenate([f64, f64], 0)  # [128, T]
    return np.cos(f128).astype(np.float32), np.sin(f128).astype(np.float32)


def _chunkT(w, n, p=128):
    """W [O, I] -> W.T chunked: [p, n, O] with [ki, k, :] = W[:, p*k + ki].T"""
    return np.ascontiguousarray(w.T.reshape(n, p, w.shape[0]).transpose(1, 0, 2))


def _colchunks(b, n, p=128):
    """bias [n*p] -> [p, n] with column m = chunk m."""
    return np.ascontiguousarray(b.reshape(n, p).T)


def _f8(a):
    return np.ascontiguousarray(
        np.asarray(a, np.float32).astype(ml_dtypes.float8_e4m3fn).view(np.uint8))


def _b16(a):
    return np.ascontiguousarray(
        np.asarray(a, np.float32).astype(ml_dtypes.bfloat16).view(np.uint16))


def _build_program(has_bias):
    from concourse import bacc, mybir, tile

    dt = mybir.dt
    f32 = dt.float32
    f8 = dt.float8e4
    bf = dt.bfloat16
    u8 = dt.uint8
    u16 = dt.uint16
    Alu = mybir.AluOpType
    Act = mybir.ActivationFunctionType
    DR = mybir.MatmulPerfMode.DoubleRow

    nc = bacc.Bacc(
        "TRN2",
        target_bir_lowering=False,
        debug=False,
        enable_asserts=False,
        num_devices=N_CORES,
    )

    d_x = nc.dram_tensor("x", [BPC, HIDDEN, TT], f32, kind="ExternalInput")
    d_cond = nc.dram_tensor("cond", [BPC, COND, TS], f32, kind="ExternalInput")
    d_cosq = nc.dram_tensor("cosq", [128, TT], u16, kind="ExternalInput")
    d_sinq = nc.dram_tensor("sinq", [128, TT], u16, kind="ExternalInput")
    d_cosk = nc.dram_tensor("cosk", [128, TS], u16, kind="ExternalInput")
    d_sink = nc.dram_tensor("sink", [128, TS], u16, kind="ExternalInput")
    d_wcT = nc.dram_tensor("wcT", [128, 4, 256], u8, kind="ExternalInput")
    d_wqT = nc.dram_tensor("wqT", [128, 2, 256], u8, kind="ExternalInput")
    d_wqrT = nc.dram_tensor("wqrT", [128, 2, 256], u8, kind="ExternalInput")
    d_wkT = nc.dram_tensor("wkT", [128, 2, 256], u8, kind="ExternalInput")
    d_wkrT = nc.dram_tensor("wkrT", [128, 2, 256], u8, kind="ExternalInput")
    d_wvT = nc.dram_tensor("wvT", [128, 2, 256], u8, kind="ExternalInput")
    d_wfoT = nc.dram_tensor("wfoT", [128, 2, 512], u8, kind="ExternalInput")
    d_oj = nc.dram_tensor("oj", [128, 2, 128], u8, kind="ExternalInput")
    d_bias = nc.dram_tensor("bias", [128, 14], f32, kind="ExternalInput")
    d_out = nc.dram_tensor("out", [BPC, HIDDEN, TT], f32, kind="ExternalOutput")

    with tile.TileContext(nc) as tc:
        with (
            tc.tile_pool(name="wp", bufs=1) as wp,
            tc.tile_pool(name="mp", bufs=2) as mp,
            tc.tile_pool(name="pp", bufs=2, space="PSUM") as pp,
        ):
            # ---- persistent tables / weights ----
            cosq = wp.tile([128, TT], bf)
            sinq = wp.tile([128, TT], bf)
            cosk = wp.tile([128, TS], bf)
            sink = wp.tile([128, TS], bf)
            wcT = wp.tile([128, 4, 256], f8)
            wqT = wp.tile([128, 2, 256], f8)
            wqrT = wp.tile([128, 2, 256], f8)
            wkT = wp.tile([128, 2, 256], f8)
            wkrT = wp.tile([128, 2, 256], f8)
            wvT = wp.tile([128, 2, 256], f8)
            wfoT = wp.tile([128, 2, 512], f8)
            oj = wp.tile([128, 2, 128], f8)
            bias = wp.tile([128, 14], f32)
            def load_weights_early():
                # everything the cond-side convs need, loaded first
                for t, d in [
                    (bias, d_bias), (wcT, d_wcT), (wkT, d_wkT), (wkrT, d_wkrT),
                    (cosk, d_cosk), (sink, d_sink), (wvT, d_wvT), (oj, d_oj),
                ]:
                    nc.sync.dma_start(t[:], d[:].bitcast(t.dtype))

            def load_weights_late():
                for t, d in [
                    (wqT, d_wqT), (wqrT, d_wqrT), (cosq, d_cosq), (sinq, d_sinq),
                    (wfoT, d_wfoT),
                ]:
                    nc.sync.dma_start(t[:], d[:].bitcast(t.dtype))
            # bias columns: 0,1 bc | 2,3 bq | 4,5 bqr | 6,7 bk | 8,9 bkr
            #               10,11 b2gamma | 12,13 b2beta
            bc_ = lambda m: bias[:, 0 + m : 1 + m]
            bq_ = lambda m: bias[:, 2 + m : 3 + m]
            bqr_ = lambda m: bias[:, 4 + m : 5 + m]
            bk_ = lambda m: bias[:, 6 + m : 7 + m]
            bkr_ = lambda m: bias[:, 8 + m : 9 + m]
            bfg_ = lambda m: bias[:, 10 + m : 11 + m]
            bfb_ = lambda m: bias[:, 12 + m : 13 + m]

            st = [dict() for _ in range(BPC)]  # per-batch tile state

            def load_cond(b):
                s = st[b]
                c32 = mp.tile([128, 4, TS], f32, tag="c32", bufs=2, name=f"c32{b}")
                for kk in range(4):
                    nc.sync.dma_start(
                        c32[:, kk, :], d_cond[b, kk * 128 : kk * 128 + 128, :]
                    )
                cf = mp.tile([128, 4, TS], f8, tag="condf8", bufs=2, name=f"condf8{b}")
                nc.gpsimd.tensor_copy(cf[:], c32[:])
                s["condf8"] = cf

            def load_x(b):
                s = st[b]
                s["x32"] = []
                for ch in range(2):
                    xt = mp.tile([128, TT], f32, tag="x", bufs=4, name=f"x{b}{ch}")
                    nc.sync.dma_start(xt[:], d_x[b, ch * 128 : ch * 128 + 128, :])
                    s["x32"].append(xt)
                xf8 = mp.tile([128, 2, TT], f8, tag="xf8", bufs=2, name=f"xf8{b}")
                for ch in range(2):
                    nc.gpsimd.tensor_copy(xf8[:, ch, :], s["x32"][ch][:])
                s["xf8"] = xf8

            def conv_c(b):
                s = st[b]
                cf8 = mp.tile([128, 2, TS], f8, tag="c", bufs=2, name=f"c{b}")
                for m in range(2):
                    ps = pp.tile([128, 512], f32, tag="cv", bufs=2, name=f"psc{b}{m}")
                    for kp in range(2):
                        nc.tensor.matmul(
                            ps[:],
                            wcT[:, 2 * kp : 2 * kp + 2, m * 128 : m * 128 + 128],
                            s["condf8"][:, 2 * kp : 2 * kp + 2, :],
                            start=(kp == 0), stop=(kp == 1), perf_mode=DR,
                        )
                    # cf8 = WS*c + WS*bc  (bias column pre-scaled by WS on host)
                    nc.vector.tensor_scalar_add(cf8[:, m, :], ps[:], bc_(m))
                s["cf8"] = cf8

            def conv_k(b):
                s = st[b]
                s["krope"] = []
                for m in range(2):
                    psk = pp.tile([128, 512], f32, tag="cv", bufs=2, name=f"psk{b}{m}")
                    pskr = pp.tile([128, 512], f32, tag="cv", bufs=2, name=f"pskr{b}{m}")
                    nc.tensor.matmul(
                        psk[:], wkT[:, :, m * 128 : m * 128 + 128], s["cf8"][:],
                        start=True, stop=True, perf_mode=DR,
                    )
                    nc.tensor.matmul(
                        pskr[:], wkrT[:, :, m * 128 : m * 128 + 128], s["cf8"][:],
                        start=True, stop=True, perf_mode=DR,
                    )
                    t1 = mp.tile([128, TS], bf, tag="kt1", bufs=2, name=f"kt1{b}{m}")
                    t2 = mp.tile([128, TS], bf, tag="kt2", bufs=2, name=f"kt2{b}{m}")
                    # (64*k + 64*bk) * (cos/64) == k_rope exactly
                    nc.vector.scalar_tensor_tensor(
                        t1[:], psk[:], bk_(m), cosk[:], op0=Alu.add, op1=Alu.mult
                    )
                    nc.vector.scalar_tensor_tensor(
                        t2[:], pskr[:], bkr_(m), sink[:], op0=Alu.add, op1=Alu.mult
                    )
                    kr = mp.tile([128, TS], bf, tag="krope", bufs=2, name=f"krope{b}{m}")
                    nc.gpsimd.tensor_add(kr[:], t1[:], t2[:])
                    s["krope"].append(kr)

            def conv_vt(b):
                s = st[b]
                # block-diagonal v^T: [s128, sc4, hp2, j2, 128]
                vt = mp.tile([128, 4, 2, 2, 128], f8, tag="vt", bufs=2, name=f"vt{b}")
                nc.gpsimd.memset(vt[:, :, :, 0, 64:128], 0.0)
                nc.gpsimd.memset(vt[:, :, :, 1, 0:64], 0.0)
                for sc in range(4):
                    ps = pp.tile([128, 4, 128], f32, tag="cv", bufs=2, name=f"psvt{b}{sc}")
                    nc.tensor.matmul(
                        ps[:, 0:2, :],
                        s["cf8"][:, :, sc * 128 : sc * 128 + 128], wvT[:],
                        start=True, stop=True, perf_mode=DR,
                    )
                    # j=0 rows: heads 0,2 (cols 0:64 of each pair)
                    nc.vector.tensor_copy(vt[:, sc, :, 0, 0:64], ps[:, 0:2, 0:64])
                    nc.vector.tensor_copy(vt[:, sc, :, 1, 64:128], ps[:, 0:2, 64:128])
                s["vt"] = vt

            def conv_q(b, nb):
                """q/qr conv + rope for one 512-wide t-chunk (feeds attn(b, nb))."""
                s = st[b]
                if nb == 0:
                    s["qrope"] = [
                        mp.tile([128, 4, 512], bf, tag="qrope", bufs=4, name=f"qrope{b}{m}")
                        for m in range(2)
                    ]
                sl = slice(nb * 512, nb * 512 + 512)
                for m in range(2):
                    psq = pp.tile([128, 512], f32, tag="cv", bufs=2, name=f"psq{b}{m}{nb}")
                    psqr = pp.tile([128, 512], f32, tag="cv", bufs=2, name=f"psqr{b}{m}{nb}")
                    nc.tensor.matmul(
                        psq[:], wqT[:, :, m * 128 : m * 128 + 128],
                        s["xf8"][:, :, nb * 512 : nb * 512 + 512],
                        start=True, stop=True, perf_mode=DR,
                    )
                    nc.tensor.matmul(
                        psqr[:], wqrT[:, :, m * 128 : m * 128 + 128],
                        s["xf8"][:, :, nb * 512 : nb * 512 + 512],
                        start=True, stop=True, perf_mode=DR,
                    )
                    t1 = mp.tile([128, 512], bf, tag="qt1", bufs=2, name=f"qt1{b}{m}{nb}")
                    nc.vector.scalar_tensor_tensor(
                        t1[:], psq[:], bq_(m), cosq[:, sl], op0=Alu.add, op1=Alu.mult
                    )
                    t2 = mp.tile([128, 512], bf, tag="qt2", bufs=2, name=f"qt2{b}{m}{nb}")
                    nc.vector.scalar_tensor_tensor(
                        t2[:], psqr[:], bqr_(m), sinq[:, sl], op0=Alu.add, op1=Alu.mult
                    )
                    nc.gpsimd.tensor_add(s["qrope"][m][:, nb, :], t1[:], t2[:])

            def attn(b, tq):
                s = st[b]
                if tq == 0:
                    s["ntp"] = {}
                ntp = mp.tile([128, 2, 512], f8, tag="ntp", bufs=3, name=f"ntp{b}{tq}")
                s["ntp"][tq] = ntp
                ps_p = {}
                for hp in range(2):
                    # p tile [s128, head-in-pair 2, sc 4, t 512]
                    p = mp.tile([128, 2, 4, 512], f8, tag="p", bufs=3, name=f"p{b}{tq}{hp}")
                    ps_p[hp] = p
                for h in range(H):
                    hp, hh = divmod(h, 2)
                    chq = h // 2
                    base = 64 * (h % 2)
                    p = ps_p[hp]
                    for e in range(2):
                        pss = pp.tile([128, 2, 512], f32, tag="pss", bufs=2,
                                      name=f"pss{b}{tq}{h}{e}")
                        for j in range(2):
                            sc = 2 * e + j
                            nc.tensor.matmul(
                                pss[:, j, :],
                                s["krope"][chq][base : base + 64, sc * 128 : sc * 128 + 128],
                                s["qrope"][chq][base : base + 64, tq, :],
                                start=True, stop=True,
                            )
                        nc.scalar.activation(
                            p[:, hh, 2 * e : 2 * e + 2, :], pss[:], Act.Exp, scale=0.125
                        )
                    if h % 2 == 1:
                        # pair complete: attention out + Z + normalize
                        p = ps_p[hp]
                        pso = pp.tile([128, 512], f32, tag="att", bufs=2,
                                      name=f"pso{b}{tq}{hp}")
                        for sc in range(4):
                            nc.tensor.matmul(
                                pso[:], s["vt"][:, sc, hp, :, :], p[:, :, sc, :],
                                start=(sc == 0), stop=(sc == 3), perf_mode=DR,
                            )
                        zb = pp.tile([128, 512], f32, tag="att", bufs=2,
                                     name=f"zb{b}{tq}{hp}")
                        for sc in range(4):
                            nc.tensor.matmul(
                                zb[:], oj[:], p[:, :, sc, :],
                                start=(sc == 0), stop=(sc == 3), perf_mode=DR,
                            )
                        zr = mp.tile([128, 512], f32, tag="zr", bufs=2,
                                     name=f"zr{b}{tq}{hp}")
                        nc.vector.reciprocal(zr[:], zb[:])
                        # ntp = pso * zr = (64*a_hat) / (4*Z) = 16*a
                        nc.vector.tensor_mul(ntp[:, hp, :], pso[:], zr[:])

            def film(b, tq, beta_on_act):
                s = st[b]
                if tq == 0:
                    s["ft"] = []
                    for ch in range(2):
                        ft = mp.tile([128, TT], f32, tag="ft", bufs=4, name=f"ft{b}{ch}")
                        s["ft"].append(ft)
                ntp = s["ntp"][tq]
                tsl = slice(tq * 512, tq * 512 + 512)
                for ch in range(2):
                    psf = pp.tile([128, 2, 512], f32, tag="pss", bufs=2,
                                  name=f"psf{b}{tq}{ch}")
                    nc.tensor.matmul(
                        psf[:, 0, :], wfoT[:, :, ch * 128 : ch * 128 + 128], ntp[:],
                        start=True, stop=True, perf_mode=DR,
                    )
                    nc.tensor.matmul(
                        psf[:, 1, :], wfoT[:, :, 256 + ch * 128 : 256 + ch * 128 + 128],
                        ntp[:],
                        start=True, stop=True, perf_mode=DR,
                    )
                    # tg = (psf_gamma/128) * x ; psf = (8wfo)@(16a) = 128*gamma
                    tg = mp.tile([128, 512], f32, tag="tg", bufs=2, name=f"tg{b}{tq}{ch}")
                    nc.vector.scalar_tensor_tensor(
                        tg[:], psf[:, 0, :], 1.0 / 128.0, s["x32"][ch][:, tsl],
                        op0=Alu.mult, op1=Alu.mult,
                    )
                    ftsl = s["ft"][ch][:, tsl]
                    if beta_on_act:
                        # evict beta on the scalar engine, add on Pool (both
                        # off the DVE critical path)
                        fb = mp.tile([128, 512], f32, tag="fb", bufs=2,
                                     name=f"fb{b}{tq}{ch}")
                        if has_bias:
                            nc.scalar.activation(
                                fb[:], psf[:, 1, :], Act.Identity,
                                bias=bfb_(ch), scale=1.0 / 128.0,
                            )
                        else:
                            nc.scalar.activation(
                                fb[:], psf[:, 1, :], Act.Copy, scale=1.0 / 128.0,
                            )
                        nc.gpsimd.tensor_add(ftsl, fb[:], tg[:])
                    else:
                        nc.vector.scalar_tensor_tensor(
                            ftsl, psf[:, 1, :], 1.0 / 128.0, tg[:],
                            op0=Alu.mult, op1=Alu.add,
                        )
                        if has_bias:
                            nc.vector.tensor_scalar_add(ftsl, ftsl, bfb_(ch))
                    if has_bias:
                        # y += x*b2gamma (skipped when biases are zero)
                        nc.vector.scalar_tensor_tensor(
                            ftsl, s["x32"][ch][:, tsl], bfg_(ch), ftsl,
                            op0=Alu.mult, op1=Alu.add,
                        )
                    nc.sync.dma_start(
                        d_out[b, ch * 128 : ch * 128 + 128, tsl], ftsl
                    )

            # ---- emission schedule: per-t-chunk pipeline, overlap batch-1
            # convs with batch-0 attention ----
            load_weights_early()
            load_cond(0)
            load_x(0)
            load_weights_late()
            conv_c(0); conv_k(0); conv_vt(0)
            conv_q(0, 0); attn(0, 0)
            load_cond(1); load_x(1)
            conv_q(0, 1); attn(0, 1); film(0, 0, True)
            conv_q(0, 2); attn(0, 2); film(0, 1, True); conv_c(1)
            conv_q(0, 3); attn(0, 3); film(0, 2, True); conv_k(1); conv_vt(1)
            conv_q(1, 0); attn(1, 0); film(0, 3, True)
            conv_q(1, 1); attn(1, 1); film(1, 0, False)
            conv_q(1, 2); attn(1, 2); film(1, 1, False)
            conv_q(1, 3); attn(1, 3); film(1, 2, False)
            film(1, 3, False)

    nc.compile()
    return nc


def _host_prep(inputs):
    wq, bq = inputs["wq"], inputs["bq"]
    wk, bk = inputs["wk"], inputs["bk"]
    wv, bv = inputs["wv"], inputs["bv"]
    wc, bc = inputs["w_cond"], inputs["b_cond"]
    wo = inputs["wo"]
    wf, bf_ = inputs["w_film"], inputs["b_film"]

    cosq, sinq = _rope_tables(TT)
    cosk, sink = _rope_tables(TS)
    # fold output projection and bv/bo into the film conv (host fp64)
    wfo = (wf.astype(np.float64) @ wo.astype(np.float64)).astype(np.float32)
    b2 = (
        wfo.astype(np.float64) @ bv.astype(np.float64)
        + wf.astype(np.float64) @ inputs["bo"].astype(np.float64)
        + bf_
    ).astype(np.float32)

    ojv = np.zeros((128, 2, 128), np.float32)
    ojv[:, 0, 0:64] = 4.0
    ojv[:, 1, 64:128] = 4.0

    bias = np.zeros((128, 14), np.float32)
    bias[:, 0:2] = _colchunks(bc, 2) * WS
    bias[:, 2:4] = _colchunks(bq, 2) * WS
    bias[:, 4:6] = _colchunks(_rot_fold(bq[:, None])[:, 0], 2) * WS
    bias[:, 6:8] = _colchunks(bk, 2) * WS * WS
    bias[:, 8:10] = _colchunks(_rot_fold(bk[:, None])[:, 0], 2) * WS * WS
    bias[:, 10:12] = _colchunks(b2[:HIDDEN], 2)
    bias[:, 12:14] = _colchunks(b2[HIDDEN:], 2)

    shared = {
        "cosq": _b16(cosq / WS), "sinq": _b16(sinq / WS),
        "cosk": _b16(cosk / (WS * WS)), "sink": _b16(sink / (WS * WS)),
        "wcT": _f8(_chunkT(wc, 4) * WS),
        "wqT": _f8(_chunkT(wq, 2) * WS),
        "wqrT": _f8(_chunkT(_rot_fold(wq), 2) * WS),
        "wkT": _f8(_chunkT(wk, 2) * WS),
        "wkrT": _f8(_chunkT(_rot_fold(wk), 2) * WS),
        "wvT": _f8(_chunkT(wv, 2) * WS),
        "wfoT": _f8(_chunkT(wfo, 2) * WS),
        "oj": _f8(ojv),
        "bias": np.ascontiguousarray(bias),
    }
    has_bias = bool(np.any(b2 != 0.0))
    return shared, has_bias


def kernel(**inputs):
    from concourse.bass_utils import run_bass_kernel_spmd

    inputs = {k: np.asarray(v, dtype=np.float32) for k, v in inputs.items()}
    shared, has_bias = _host_prep(inputs)

    key = ("nc", has_bias)
    if key not in _CACHE:
        _CACHE["nc"] = _build_program(has_bias)
        _CACHE[key] = _CACHE["nc"]
    nc = _CACHE[key]

    x = inputs["x"]
    cond = inputs["cond_latent"]
    in_maps = []
    for c in range(N_CORES):
        m = dict(shared)
        m["x"] = np.ascontiguousarray(x[c * BPC : (c + 1) * BPC])
        m["cond"] = np.ascontiguousarray(cond[c * BPC : (c + 1) * BPC])
        in_maps.append(m)

    res = run_bass_kernel_spmd(nc, in_maps, list(range(N_CORES)))
    out = np.concatenate([res.results[c]["out"] for c in range(N_CORES)], axis=0)
    return out.astype(np.float32)
